# revision 14
# baseline (speedup 1.0000x reference)
"""Trainium2 Bass kernel for nn_ActionDecoder (MoE-routed 2-layer GELU MLP).

Problem: per batch row b (2048 rows x 16 timesteps), route through the
embodiment_ids[b]-th expert MLP: out = GELU(x @ W1[e] + b1[e]) @ W2[e] + b2[e].
x: [2048, 16, 512] f32, W1: [4, 512, 1024], W2: [4, 1024, 28].

Strategy (expert-parallel): host sorts batch rows by embodiment, gives each of
the 8 cores one expert (2 cores per expert, half the expert's rows each). Each
core runs a dense 2-layer MLP over its tokens with its own expert's weights
(weights are per-core *data*, so one SPMD program serves all cores). Activations
are fed transposed ([d, tok]) so both matmuls keep weights stationary; compute
in bf16 with fp32 PSUM accumulation.

Perf notes:
- Token dim tiled as 512-token tiles plus one 128-multiple remainder tile so
  SPMD padding is at most 127 tokens.
- Layer 2 (M=28) packs 4 h-chunks into the 4 PE column groups concurrently
  (tile_position), then combines the 4 PSUM partition strips on DVE.
- A few dependency-free warmup matmuls run during the initial DMA wait to
  lift the PE HAM clock gate to 8/8 before real work arrives.
"""

import numpy as np
import ml_dtypes

import concourse.bass as bass
import concourse.bacc as bacc
import concourse.mybir as mybir
from concourse.tile import TileContext
from concourse.bass_utils import run_bass_kernel_spmd

# Model dims (hardcoded per problem spec)
D = 512      # d_model
H = 1024     # hidden
A = 28       # max action dim
E = 4        # n embodiments
N_CORES = 8
P = 128      # partitions
TILE = 512   # main token tile
GRAIN = 128  # token granularity (min tile)
KC = D // P  # 4 contraction chunks for layer 1
HC = H // P  # 8 hidden chunks

N_WARMUP_MM = 2   # dependency-free matmuls to warm the PE clock gate
PACK_L2 = True     # pack layer-2 into PE column groups

F32 = mybir.dt.float32
BF16 = mybir.dt.bfloat16

_PROGRAM_CACHE = {}

# Set by test harness to collect a profile: None | dict (filled with results)
TRACE_SINK = None


def _tile_sizes(ntok):
    sizes = [TILE] * (ntok // TILE)
    if ntok % TILE:
        sizes.append(ntok % TILE)  # remainder last: short pipeline tail
    return sizes


def _build_program(ntok):
    assert ntok % GRAIN == 0
    sizes = _tile_sizes(ntok)
    nc = bacc.Bacc()

    x_in = nc.declare_dram_parameter("x", [P, KC, ntok], BF16, isOutput=False)
    w1_in = nc.declare_dram_parameter("w1", [P, HC, KC, P], BF16, isOutput=False)
    w2_in = nc.declare_dram_parameter("w2", [P, HC, A], BF16, isOutput=False)
    b1_in = nc.declare_dram_parameter("b1", [P, HC], F32, isOutput=False)
    b2_in = nc.declare_dram_parameter("b2", [A, 1], F32, isOutput=False)
    out = nc.declare_dram_parameter("out", [A, ntok], F32, isOutput=True)

    with TileContext(nc) as tc:
        with (
            tc.tile_pool(name="wpool", bufs=1) as wpool,
            tc.tile_pool(name="xpool", bufs=4) as xpool,
            tc.tile_pool(name="hpool", bufs=3) as hpool,
            tc.tile_pool(name="opool", bufs=3) as opool,
            tc.tile_pool(name="ps_h", bufs=6, space="PSUM") as ps_h_pool,
            tc.tile_pool(name="ps_o", bufs=2, space="PSUM") as ps_o_pool,
        ):
            # --- PE warmup: no data deps, runs during the initial DMA wait ---
            warm_x = wpool.tile([P, TILE], BF16)
            nc.gpsimd.memset(warm_x, 0.0)
            warm_ps = ps_h_pool.tile([P, TILE], F32, tag="ps_h")
            for _ in range(N_WARMUP_MM):
                nc.tensor.matmul(warm_ps, warm_x[:, :P], warm_x,
                                 start=True, stop=True)

            # --- Weight/x loads, interleaved 128KB pieces so the first
            # matmuls start as early as possible ---
            w1_sb = wpool.tile([P, HC, KC, P], BF16)
            x_sb0 = xpool.tile([P, KC, sizes[0]], BF16, tag="x")
            b1_sb = wpool.tile([P, HC], F32)
            b2_sb = wpool.tile([A, 1], F32)
            # x via the scalar engine's HWDGE queue, w1 via sync's: two
            # queues stream in parallel at kernel start.
            for kc in range(KC):
                nc.scalar.dma_start(out=x_sb0[:, kc], in_=x_in[:, kc, 0:sizes[0]])
            for hc in range(HC):
                nc.sync.dma_start(out=w1_sb[:, hc], in_=w1_in[:, hc])
            nc.gpsimd.dma_start(out=b1_sb, in_=b1_in[:])
            nc.gpsimd.dma_start(out=b2_sb, in_=b2_in[:])
            w2_sb = wpool.tile([P, HC, A], BF16)
            nc.gpsimd.dma_start(out=w2_sb, in_=w2_in[:])
            x_sb1 = None
            if len(sizes) > 1:
                x_sb1 = xpool.tile([P, KC, sizes[1]], BF16, tag="x")
                nc.scalar.dma_start(out=x_sb1, in_=x_in[:, :, sizes[0]:sizes[0] + sizes[1]])

            def emit_l2(h_sb, off, size, packed):
                """Layer 2: out[:, off:off+size] = W2^T h + b2."""
                o_sb = opool.tile([A, size], F32, tag="o")
                if packed:
                    # 4 h-chunks run concurrently in the 4 PE column groups,
                    # accumulating 2 rounds; strips combined on DVE (which may
                    # read at most one PSUM operand per instruction).
                    o_ps = ps_o_pool.tile([P, size], F32, tag="ps_o")
                    for r in range(2):
                        for j in range(4):
                            hc = r * 4 + j
                            nc.tensor.matmul(
                                o_ps[32 * j:32 * j + A, :],
                                w2_sb[:, hc],
                                h_sb[:, hc],
                                start=(r == 0),
                                stop=(r == 1),
                                tile_position=(0, 32 * j),
                            )
                    nc.vector.tensor_scalar_add(o_sb, o_ps[0:A], b2_sb)
                    nc.vector.tensor_add(o_sb, o_sb, o_ps[32:32 + A])
                    nc.vector.tensor_add(o_sb, o_sb, o_ps[64:64 + A])
                    nc.vector.tensor_add(o_sb, o_sb, o_ps[96:96 + A])
                else:
                    o_ps = ps_o_pool.tile([A, size], F32, tag="ps_o")
                    for hc in range(HC):
                        nc.tensor.matmul(
                            o_ps,
                            w2_sb[:, hc],
                            h_sb[:, hc],
                            start=(hc == 0),
                            stop=(hc == HC - 1),
                        )
                    nc.vector.tensor_scalar_add(o_sb, o_ps, b2_sb)
                nc.sync.dma_start(out=out[:, off:off + size], in_=o_sb)

            # Layer 2 for tile t is emitted after layer 1 of tile t+1 so its
            # matmuls never wait on a just-finished GELU (PE is in-order).
            # The last two tiles use unpacked L2: in the drain tail PE is
            # idle anyway and the single-op DVE epilogue is shorter.
            pend = None
            off = 0
            for t, size in enumerate(sizes):
                if t == 0:
                    x_sb = x_sb0
                elif t == 1:
                    x_sb = x_sb1
                else:
                    x_sb = xpool.tile([P, KC, size], BF16, tag="x")
                    nc.sync.dma_start(out=x_sb, in_=x_in[:, :, off:off + size])

                # --- Layer 1: h = gelu(W1^T x + b1), per 128-row h-chunk ---
                h_sb = hpool.tile([P, HC, size], BF16, tag="h")
                for hc in range(HC):
                    ps = ps_h_pool.tile([P, size], F32, tag="ps_h")
                    for kc in range(KC):
                        nc.tensor.matmul(
                            ps,
                            w1_sb[:, hc, kc],
                            x_sb[:, kc],
                            start=(kc == 0),
                            stop=(kc == KC - 1),
                        )
                    nc.scalar.activation(
                        h_sb[:, hc], ps,
                        mybir.ActivationFunctionType.Gelu,
                        bias=b1_sb[:, hc:hc + 1],
                    )

                if pend is not None:
                    packed = PACK_L2 and pend[3] < len(sizes) - 2
                    emit_l2(pend[0], pend[1], pend[2], packed)
                pend = (h_sb, off, size, t)
                off += size

            packed = PACK_L2 and pend[3] < len(sizes) - 2
            emit_l2(pend[0], pend[1], pend[2], packed)

    nc.finalize()
    return nc


def kernel(pred_action_latents, W1, b1, W2, b2, embodiment_ids):
    x = np.asarray(pred_action_latents)
    W1 = np.asarray(W1)
    b1 = np.asarray(b1)
    W2 = np.asarray(W2)
    b2 = np.asarray(b2)
    ids = np.asarray(embodiment_ids)

    B, T, _ = x.shape
    assert W1.shape[0] == E and N_CORES == 2 * E

    # --- Host-side routing/sharding ---
    order = np.argsort(ids, kind="stable")
    counts = np.bincount(ids, minlength=E)
    starts = np.concatenate([[0], np.cumsum(counts)])

    # core 2e, 2e+1 handle expert e (first/second half of its rows)
    core_rows = []
    for e in range(E):
        rows_e = order[starts[e]:starts[e + 1]]
        h1 = (len(rows_e) + 1) // 2
        core_rows.append(rows_e[:h1])
        core_rows.append(rows_e[h1:])

    max_tok = max(len(r) * T for r in core_rows)
    ntok = max(GRAIN, ((max_tok + GRAIN - 1) // GRAIN) * GRAIN)

    if ntok not in _PROGRAM_CACHE:
        _PROGRAM_CACHE[ntok] = _build_program(ntok)
    nc = _PROGRAM_CACHE[ntok]

    in_maps = []
    for c in range(N_CORES):
        e = c // 2
        rows = core_rows[c]
        ntok_real = len(rows) * T
        xr = np.zeros((ntok, D), dtype=np.float32)
        xr[:ntok_real] = x[rows].reshape(ntok_real, D)
        # [P, KC, ntok]: (p, kc, n) = xr[n, kc*P+p]
        x_dev = np.ascontiguousarray(
            xr.reshape(ntok, KC, P).transpose(2, 1, 0)
        ).astype(ml_dtypes.bfloat16)
        # [P, HC, KC, 128]: (p, hc, kc, j) = W1[e, kc*P+p, hc*P+j]
        w1_dev = np.ascontiguousarray(
            W1[e].reshape(KC, P, HC, P).transpose(1, 2, 0, 3)
        ).astype(ml_dtypes.bfloat16)
        w2_dev = np.ascontiguousarray(
            W2[e].reshape(HC, P, A).transpose(1, 0, 2)
        ).astype(ml_dtypes.bfloat16)
        b1_dev = np.ascontiguousarray(b1[e].reshape(HC, P).T).astype(np.float32)
        b2_dev = np.ascontiguousarray(b2[e].reshape(A, 1)).astype(np.float32)
        in_maps.append({
            "x": x_dev, "w1": w1_dev, "w2": w2_dev, "b1": b1_dev, "b2": b2_dev,
        })

    trace = TRACE_SINK is not None
    res = run_bass_kernel_spmd(nc, in_maps, core_ids=list(range(N_CORES)),
                               trace=trace)
    if trace:
        TRACE_SINK["exec_time_ns"] = res.exec_time_ns
        TRACE_SINK["mean_exec_time_ns"] = res.mean_exec_time_ns
        TRACE_SINK["profile_json"] = res.profile_json

    # --- Host-side unshard ---
    out_full = np.zeros((B, T, A), dtype=np.float32)
    for c in range(N_CORES):
        rows = core_rows[c]
        if len(rows) == 0:
            continue
        o = np.asarray(res.results[c]["out"])  # [A, ntok] f32
        out_full[rows] = o[:, :len(rows) * T].T.reshape(len(rows), T, A)
    return out_full
